# revision 15
# baseline (speedup 1.0000x reference)
"""Trainium2 Bass kernel for ContrastiveHessianCalculator GGN-diagonal.

Math (see the reference docstring):
  out = concat([W1d.flat, b1d, W2d.flat, b2d])   # [164416]
  c_i = sum_o W2[o,i]^2
  For a pair batch (ia, ib):
    h = tanh(x @ W1.T + b1); d = 1 - h^2 (per side a/b)
    W1d[i,j] = c_i * sum_p (da^2 xa_j^2 - 2 da db xa_j xb_j + db^2 xb_j^2)
    b1d[i]   = c_i * sum_p (da^2 - 2 da db + db^2)
    W2d[o,i] = sum_p (ha - hb)^2   (same for every o);  b2d = 0
  out = pos-pairs - neg-pairs.

Sharding: data-parallel over the pair dim P across 8 cores (128 pairs per
core per pos/neg block).  Each core's shard of x is the set of rows its
pair indices reference (the "all-gathered rows" option from the sharding
hint), staged in both layouts the kernel needs: pair-major (for the V
x-product tiles) and feature-major, packed next to the transposed W1
chunk it multiplies (one DMA per d-chunk feeds the z matmul directly, no
on-device transposes).

The p-sum is one accumulated matmul per h-chunk: U k-tiles
[4*da^2, 4*da*db, 4*db^2] (pos and neg, from d2 = 2 - 2*h^2) against V
k-tiles [scaled x-products, b1 col]; the scale/sign constants (+-1/4,
-+1/2) live in the V tensor-scalar prescales and ones-column memsets, so
neither block needs a negation pass.  The (ha-hb)^2 column comes from
1-column matmuls against +-1 vectors.  z runs in bf16 (PE full rate);
all post-tanh elementwise work is bf16 on DVE/ACT/Pool.

Finish: AllReduce is expensive; each core DMAs its [128, 4, 258] bf16
partial to DRAM, one ReduceScatter sums over the 8 cores, and the host
concatenates the 8 disjoint shards (pure layout, no arithmetic).
"""

import numpy as np
import ml_dtypes

import concourse.bass as bass
import concourse.tile as tile
from concourse import bacc, bass_utils, mybir

F32 = mybir.dt.float32
BF16 = mybir.dt.bfloat16
AF = mybir.ActivationFunctionType
ALU = mybir.AluOpType

N, D, H, O, P = 50000, 256, 512, 64, 1024
NCORES = 8
PP = P // NCORES          # 128 pairs per core per pos/neg block
HC = H // 128             # 4 h-chunks
DC = D // 128             # 2 d-chunks
NPARAM = H * D + H + O * H + O  # 164416
VW = D + 2                # output row: 256 W1d cols + b1d col + hd col
VB = D + 1                # S-matmul rhs width (x-products + b1 col)
NJUNK = 8                 # PE warmup chain length

_CACHE = {}


def _build_program():
    nc = bacc.Bacc(
        "TRN2",
        debug=False,
        enable_asserts=False,
        target_bir_lowering=False,
        num_devices=NCORES,
    )
    # z inputs packed per d-chunk: zin[dc] = [xt(dc) | w1t(dc)] as [128, 1024]
    #   xt[dc, d, j*128+p] = x[idx_j[p], dc*128+d];  w1t[dc, d, h] = W1[h, dc*128+d]
    zin_d = nc.dram_tensor("zin", [DC, 128, 1024], BF16, kind="ExternalInput").ap()
    # pair-major gathered x: xg[j, p, :] = x[idx_j[p], :]
    xg_d = nc.dram_tensor("xg", [4, 128, D], BF16, kind="ExternalInput").ap()
    b1_d = nc.dram_tensor("b1r", [1, H], BF16, kind="ExternalInput").ap()
    w2_d = nc.dram_tensor("W2", [O, H], F32, kind="ExternalInput").ap()
    shard_d = nc.dram_tensor(
        "shard", [128 // NCORES, HC, VW], BF16, kind="ExternalOutput"
    ).ap()

    with tile.TileContext(nc) as tc:
        _body(tc, zin_d, xg_d, b1_d, w2_d, shard_d)
    nc.compile()
    return nc


def _body(tc, zin_d, xg_d, b1_d, w2_d, shard_d):
    nc = tc.nc
    from contextlib import ExitStack

    ctx = ExitStack()
    sg = ctx.enter_context(tc.tile_pool(name="sg", bufs=1))
    ps_z = ctx.enter_context(tc.tile_pool(name="ps_z", bufs=2, space="PSUM"))
    ps_w = ctx.enter_context(tc.tile_pool(name="ps_w", bufs=4, space="PSUM"))
    dram = ctx.enter_context(tc.tile_pool(name="dram", bufs=1, space="DRAM"))

    # ---- Pool: junk memset first (PE decoy waits on it), then constants ----
    junk = sg.tile([128, 256], BF16)
    nc.gpsimd.memset(junk[:], 0.25)
    ones_r = sg.tile([1, 128], BF16)
    nc.gpsimd.memset(ones_r[:], 1.0)
    ones64 = sg.tile([O, 1], F32)
    nc.gpsimd.memset(ones64[:], 1.0)
    vhd_p = sg.tile([128, 1], BF16)
    nc.gpsimd.memset(vhd_p[:], 1.0)
    vhd_n = sg.tile([128, 1], BF16)
    nc.gpsimd.memset(vhd_n[:], -1.0)

    # V tiles [128, 257] bf16: cols 0..255 scaled x-products, col 256 b1d col.
    # U tiles carry 4*da^2 / 4*da*db / 4*db^2, so V scales are +-1/4, -+1/2.
    v_aa_p = sg.tile([128, VB], BF16)
    v_ab_p = sg.tile([128, VB], BF16)
    v_bb_p = sg.tile([128, VB], BF16)
    v_aa_n = sg.tile([128, VB], BF16)
    v_ab_n = sg.tile([128, VB], BF16)
    v_bb_n = sg.tile([128, VB], BF16)
    for v, s in ((v_aa_p, 0.25), (v_ab_p, -0.5), (v_bb_p, 0.25),
                 (v_aa_n, -0.25), (v_ab_n, 0.5), (v_bb_n, -0.25)):
        nc.gpsimd.memset(v[:, D : D + 1], s)

    # ---- SP queue: z inputs first, then b1 (tiny), then xg ----
    zin_sb = sg.tile([128, DC, 1024], BF16)
    nc.sync.dma_start(out=zin_sb[:, 0, :], in_=zin_d[0])
    nc.sync.dma_start(out=zin_sb[:, 1, :], in_=zin_d[1])
    b1row = sg.tile([1, H], BF16)
    nc.sync.dma_start(out=b1row[:], in_=b1_d[:])
    xg_sb = sg.tile([128, 4, D], BF16)
    nc.sync.dma_start(out=xg_sb[:], in_=xg_d.rearrange("j p d -> p j d"))

    # ---- Pool/SWDGE queue: w2 ----
    w2_sb = sg.tile([O, H], F32)
    nc.gpsimd.dma_start(out=w2_sb[:], in_=w2_d[:])

    # ---- PE warmup: decoy's wait on the junk memset pins pe_busy_start at
    # ~0.4us; the chain keeps the engine busy until the z inputs land, so
    # real matmuls decode inside a >3us-old busy stretch (full rate) ----
    jp = ps_w.tile([128, 512], F32, tag="wp", name="junkp")
    nc.tensor.matmul(
        jp[:1, :1], lhsT=junk[:1, :1], rhs=junk[:1, :1], start=True, stop=True,
    )
    for i in range(NJUNK):
        jpi = ps_w.tile([128, 512], F32, tag="wp", name=f"junk{i}")
        nc.tensor.matmul(
            jpi[:, :256], lhsT=junk[:, :128], rhs=junk[:],
            start=True, stop=True,
        )

    # ---- z = xg @ W1.T + b1 accumulated in PSUM; per-j close-out.
    # Two 2-bank tiles so tanh can read a j-pair in one ACT op. ----
    zp01 = ps_z.tile([128, 2, H], F32, tag="z", name="zp01")
    zp23 = ps_z.tile([128, 2, H], F32, tag="z", name="zp23")
    zp = [zp01[:, 0, :], zp01[:, 1, :], zp23[:, 0, :], zp23[:, 1, :]]
    for j in range(4):
        nc.tensor.matmul(
            zp[j], lhsT=zin_sb[:, 0, j * 128 : (j + 1) * 128],
            rhs=zin_sb[:, 0, 512:1024], start=True, stop=False,
        )
    for j in range(4):
        nc.tensor.matmul(
            zp[j], lhsT=zin_sb[:, 1, j * 128 : (j + 1) * 128],
            rhs=zin_sb[:, 1, 512:1024], start=False, stop=False,
        )
        nc.tensor.matmul(
            zp[j], lhsT=ones_r[:], rhs=b1row[:],
            start=False, stop=True,
        )

    # ---- c = colsum(W2^2): Pool squares W2, PE sums columns into ONE bank ----
    w2sq = sg.tile([O, H], F32)
    nc.gpsimd.tensor_mul(w2sq[:], w2_sb[:], w2_sb[:])
    cpall = ps_w.tile([128, 512], F32, tag="wp", name="cpall")
    for hc in range(HC):
        nc.tensor.matmul(
            cpall[:, hc : hc + 1], lhsT=w2sq[:, hc * 128 : (hc + 1) * 128],
            rhs=ones64[:], start=(hc == 0), stop=(hc == HC - 1),
        )

    # ---- tanh -> bf16, one ACT op per j-pair ----
    ha = sg.tile([128, 4, H], BF16)
    nc.scalar.activation(out=ha[:, 0:2, :], in_=zp01[:], func=AF.Tanh)
    nc.scalar.activation(out=ha[:, 2:4, :], in_=zp23[:], func=AF.Tanh)

    # ---- V x-product tiles ----
    xa_p, xb_p = xg_sb[:, 0, :], xg_sb[:, 1, :]
    xa_n, xb_n = xg_sb[:, 2, :], xg_sb[:, 3, :]
    # DVE: the k=0 tile (vaa_p) early, plus the neg prescales in the tanh gap
    qxa_p = sg.tile([128, D], BF16)
    nc.vector.tensor_scalar_mul(qxa_p[:], xa_p, 0.25)
    nc.vector.tensor_mul(v_aa_p[:, :D], xa_p, qxa_p[:])
    qxa_n = sg.tile([128, D], BF16)
    hxb_n = sg.tile([128, D], BF16)
    qxb_n = sg.tile([128, D], BF16)
    nc.vector.tensor_scalar_mul(qxa_n[:], xa_n, -0.25)
    nc.vector.tensor_scalar_mul(hxb_n[:], xb_n, 0.5)
    nc.vector.tensor_scalar_mul(qxb_n[:], xb_n, -0.25)
    # Pool: the rest of the pos V tiles
    hxb_p = sg.tile([128, D], BF16)
    qxb_p = sg.tile([128, D], BF16)
    nc.gpsimd.tensor_scalar_mul(hxb_p[:], xb_p, -0.5)
    nc.gpsimd.tensor_mul(v_ab_p[:, :D], xa_p, hxb_p[:])
    nc.gpsimd.tensor_scalar_mul(qxb_p[:], xb_p, 0.25)
    nc.gpsimd.tensor_mul(v_bb_p[:, :D], xb_p, qxb_p[:])

    # ---- U tiles per block (bf16, pair-packed where possible) ----
    # d2 = 2 - 2*h^2 for both sides in one op; U squares are (d2)^2 = 4d^2.
    u_tiles = []   # (da_sqU, dadbU, db_sqU, hdU) per block
    c_sb = sg.tile([128, HC], F32)
    for blk in range(2):
        a = ha[:, 2 * blk, :]
        b = ha[:, 2 * blk + 1, :]
        sqab = sg.tile([128, 2, H], BF16, name=f"sqab{blk}")
        nc.vector.tensor_mul(
            sqab[:], ha[:, 2 * blk : 2 * blk + 2, :], ha[:, 2 * blk : 2 * blk + 2, :]
        )
        dab2 = sg.tile([128, 2, H], BF16, name=f"dab2{blk}")
        nc.vector.tensor_scalar(dab2[:], sqab[:], -2.0, 2.0, ALU.mult, ALU.add)
        dasqs = sg.tile([128, 2, H], BF16, name=f"dasqs{blk}")
        nc.vector.tensor_mul(dasqs[:], dab2[:], dab2[:])
        dadbU = sg.tile([128, H], BF16, name=f"dadb{blk}")
        nc.vector.tensor_mul(dadbU[:], dab2[:, 0, :], dab2[:, 1, :])
        hd_s = sg.tile([128, H], BF16, name=f"hds{blk}")
        nc.gpsimd.tensor_sub(hd_s[:], a, b)
        hdU = sg.tile([128, H], BF16, name=f"hdU{blk}")
        if blk == 0:
            nc.scalar.copy(out=c_sb[:], in_=cpall[:, :HC])
        nc.scalar.activation(out=hdU[:], in_=hd_s[:], func=AF.Square)
        u_tiles.append((dasqs[:, 0, :], dadbU[:], dasqs[:, 1, :], hdU[:]))
        if blk == 0:
            # neg V products on Pool (prescales done above on DVE)
            nc.gpsimd.tensor_mul(v_aa_n[:, :D], xa_n, qxa_n[:])
            nc.gpsimd.tensor_mul(v_ab_n[:, :D], xa_n, hxb_n[:])
            nc.gpsimd.tensor_mul(v_bb_n[:, :D], xb_n, qxb_n[:])

    # ---- big matmuls: accumulate wp[hc] over 6 wide k-tiles + 2 hd cols ----
    wp = [ps_w.tile([128, 512], F32, tag="wp", name=f"wp{hc}") for hc in range(HC)]
    seq = [
        (u_tiles[0][0], v_aa_p), (u_tiles[0][1], v_ab_p), (u_tiles[0][2], v_bb_p),
        (u_tiles[1][0], v_aa_n), (u_tiles[1][1], v_ab_n), (u_tiles[1][2], v_bb_n),
    ]
    for ki, (u, v) in enumerate(seq):
        for hc in range(HC):
            nc.tensor.matmul(
                wp[hc][:, :VB], lhsT=u[:, hc * 128 : (hc + 1) * 128], rhs=v[:],
                start=(ki == 0), stop=(ki == len(seq) - 1),
            )
        if ki == 0:  # hd pos column (inside the freshly started bank)
            for hc in range(HC):
                nc.tensor.matmul(
                    wp[hc][:, VB : VB + 1],
                    lhsT=u_tiles[0][3][:, hc * 128 : (hc + 1) * 128],
                    rhs=vhd_p[:], start=False, stop=False,
                )
        if ki == 4:  # hd neg column, before the closing (stop) k-tile
            for hc in range(HC):
                nc.tensor.matmul(
                    wp[hc][:, VB : VB + 1],
                    lhsT=u_tiles[1][3][:, hc * 128 : (hc + 1) * 128],
                    rhs=vhd_n[:], start=False, stop=False,
                )

    # ---- c-scale rows (cols 0..256), copy hd col raw; DVE hc0/1, ACT hc2/3;
    # each queue then fires its own half of the cc_in DMA ----
    partial = sg.tile([128, HC, VW], BF16)
    cc_in = dram.tile([128, HC, VW], BF16)
    for hc in (0, 1):
        nc.scalar.activation(
            out=partial[:, hc, :VB], in_=wp[hc][:, :VB],
            func=AF.Copy, scale=c_sb[:, hc : hc + 1],
        )
        nc.scalar.copy(
            out=partial[:, hc, VB : VB + 1], in_=wp[hc][:, VB : VB + 1]
        )
    nc.scalar.dma_start(out=cc_in[:, 0:2, :], in_=partial[:, 0:2, :])
    for hc in (2, 3):
        nc.vector.tensor_scalar_mul(
            partial[:, hc, :VB], wp[hc][:, :VB], c_sb[:, hc : hc + 1]
        )
        nc.vector.tensor_copy(
            out=partial[:, hc, VB : VB + 1], in_=wp[hc][:, VB : VB + 1]
        )
    nc.sync.dma_start(out=cc_in[:, 2:4, :], in_=partial[:, 2:4, :])

    # ---- ReduceScatter over the 8 cores; final hop to the output tensor ----
    SH = 128 // NCORES
    rs_out = dram.tile([SH, HC, VW], BF16)
    nc.gpsimd.collective_compute(
        "ReduceScatter",
        ALU.add,
        replica_groups=[list(range(NCORES))],
        ins=[cc_in.opt()],
        outs=[rs_out.opt()],
    )
    nc.sync.dma_start(out=shard_d[:], in_=rs_out[:])
    ctx.close()


def _get_program():
    if "nc" not in _CACHE:
        _CACHE["nc"] = _build_program()
    return _CACHE["nc"]


def kernel(**inputs):
    x = np.ascontiguousarray(np.asarray(inputs["x"], dtype=np.float32))
    W1 = np.ascontiguousarray(np.asarray(inputs["W1"], dtype=np.float32))
    b1 = np.asarray(inputs["b1"], dtype=np.float32).reshape(1, H)
    W2 = np.ascontiguousarray(np.asarray(inputs["W2"], dtype=np.float32))
    iap = np.asarray(inputs["ap"], dtype=np.int32)
    ip = np.asarray(inputs["p"], dtype=np.int32)
    ian = np.asarray(inputs["an"], dtype=np.int32)
    inn = np.asarray(inputs["n"], dtype=np.int32)

    # W1 staged transposed (layout only): w1t[dc, d, h] = W1[h, dc*128+d]
    w1t = W1.T.reshape(DC, 128, H)
    b1 = np.ascontiguousarray(b1.astype(ml_dtypes.bfloat16))

    nc = _get_program()
    in_maps = []
    for i in range(NCORES):
        s = slice(i * PP, (i + 1) * PP)
        # shard of x: the rows this core's pair indices reference, staged
        # pair-major (for V tiles) and feature-major packed beside the W1
        # chunk it multiplies (the z-matmul input).
        xg4 = np.stack([x[iap[s]], x[ip[s]], x[ian[s]], x[inn[s]]])  # [4,128,256]
        xt = xg4.transpose(2, 0, 1).reshape(DC, 128, 4 * 128)
        zin = np.ascontiguousarray(
            np.concatenate([xt, w1t], axis=2).astype(ml_dtypes.bfloat16)
        )
        xg_bf = np.ascontiguousarray(xg4.astype(ml_dtypes.bfloat16))
        in_maps.append({"zin": zin, "xg": xg_bf, "b1r": b1, "W2": W2})

    res = bass_utils.run_bass_kernel_spmd(
        nc, in_maps, core_ids=list(range(NCORES))
    )
    return _assemble([res.results[c] for c in range(NCORES)])


def _assemble(per_core):
    """Pure gather/unshard: concatenate the ReduceScatter shards and the
    device-computed W2d/b2d tail into the full [164416] output."""
    shards = np.stack([np.asarray(per_core[c]["shard"], dtype=np.float32)
                       for c in range(NCORES)])  # [8,16,HC,VW]
    red = shards.transpose(2, 0, 1, 3).reshape(H, VW)  # h = hc*128 + 16c + q
    out = np.empty(NPARAM, np.float32)
    out[0 : H * D] = red[:, :D].reshape(-1)
    out[H * D : H * D + H] = red[:, D]
    base = H * D + H
    out[base : base + O * H] = np.tile(red[:, D + 1], O)  # W2d rows all equal hd
    out[base + O * H :] = 0.0  # b2d is exactly zero
    return out
